# revision 27
# baseline (speedup 1.0000x reference)
"""KNN flow interpolation (AccFlowSupervise) on 8 Trainium2 NeuronCores.

Problem: for each query point (pc0 transformed into pc1's ego frame), find the
3 nearest neighbors in pc1, inverse-distance-weight their flow vectors, and
also emit the ego-motion displacement (pose_flow).

Sharding: data-parallel over batch (B=2) x 4 query shards -> 8 cores. Each
core computes a [2048, 8192] negated-squared-distance matrix against the full
replicated reference cloud via fused PE matmuls (augmented K=5 inner dim:
-d2 = 2x.y - |x|^2 - |y|^2), packed 4-wide into PE row groups
(tile_position), then cast to bf16 on the scalar engine.

Top-3 selection per query tile [128 x 8192]:
  1. two bf16 max-fold levels (8192 -> 4096 -> 2048) on the vector engine
     (bf16 tensor_tensor runs in 2x mode);
  2. top-8 fold slots via InstMax + InstMaxIndex on the 2048-wide folded
     array; each slot covers 4 original columns. The true top-3's slots rank
     <= 3 by exact folded value (a slot's fold >= its members); 5 slots are
     kept as margin for bf16 rounding ties.
  3. one indirect DMA per slot gathers that slot's 4 candidate rows from a
     host-reordered table (ftab[s, m] = [y, |y|^2, flow, 0] of original
     column s + 2048 m) - 20 candidates;
  4. exact fp32 -d2 recompute for the candidates on GPSIMD, exact top-3 of
     20 via a tiny InstMax/InstMaxIndex (positions stay distinct for tied
     values, matching top_k), inverse-distance weights, positional one-hot
     weighted flow combine.

The tiny pose math (4x4 inverse, [N,3] point transform, q2/r2 row norms and
pose_flow = pc0_t - pc0) is done host-side in fp32, matching the reference's
formulas; the O(N*M) work all runs on-device.
"""

import os
import sys
from contextlib import ExitStack

import numpy as np

for _p in ("/opt/trn_rl_repo", "/root/.axon_site/_ro/trn_rl_repo"):
    if os.path.isdir(_p) and _p not in sys.path:
        sys.path.append(_p)

import concourse.bass as bass
import concourse.tile as tile
from concourse import mybir
from concourse.bass_utils import run_bass_kernel_spmd

# Problem shape (hardcoded; see spec)
B, N, M = 2, 8192, 8192
NCORES = 8
SHARDS = NCORES // B          # query shards per batch
NQ = N // SHARDS              # queries per core
P = 128                       # queries per tile (SBUF partitions)
NT = NQ // P                  # query tiles per core
CH = 512                      # matmul free-dim chunk (one PSUM bank fp32)
NCH = M // CH                 # chunks per tile
KA = 5                        # augmented contraction dim
NFOLD = 3                     # max-fold levels (8192 -> 1024)
NSLOT = 4                     # fold slots examined per query
NMEM = 1 << NFOLD             # original columns per fold slot
NCAND = NSLOT * NMEM          # candidates per query
MW = M >> NFOLD               # folded width
F32 = mybir.dt.float32
BF16 = mybir.dt.bfloat16
U32 = mybir.dt.uint32
EPS = 1e-8

_CACHE: dict = {}
LAST_RESULTS = None  # BassKernelResults of the most recent run (for profiling)


def _patched_drain_and_barrier(self, tick_clock, wait_clock):
    """Tile's kernel-tail drain can accumulate >2 sem waits on one Drain
    instruction, which this walrus build rejects ("Too many sync wait
    commands"). Split the waits across a chain of single-wait drains."""
    nc = self.nc
    drain_inst = nc.sync.drain()
    wait_clock.add_sem_waits(
        drain_inst.ins, tile.ScopedClock({None: tick_clock.global_clock})
    )
    si = drain_inst.ins.sync_info
    waits = list(si.on_wait or []) if si is not None else []
    if len(waits) > 1:
        si.on_wait = waits[:1]
        for w in waits[1:]:
            d2 = nc.sync.drain()
            d2.ins.sync_info = mybir.SyncInfo(on_wait=[w], on_update=[])
    nc.all_engine_barrier()
    assert self.sems is not None
    popped = nc._tile_sem_poison_stack.pop()
    assert popped is self._sem_poison
    nc.clear_and_free_semaphores(list(self.sems.allocated().values()))
    nc.all_engine_barrier()


tile.TileContext._drain_and_barrier = _patched_drain_and_barrier


def _legalize_wait_counts(nc, max_waits=1):
    """This walrus build rejects instructions carrying more than a couple of
    sem waits ("Too many sync wait commands"). Hoist extra waits onto fresh
    same-engine EventSemaphore carriers placed immediately before the
    instruction (same engine queue => identical ordering semantics)."""
    for fn in nc.m.functions:
        for bb in fn.blocks:
            out = []
            changed = False
            for ins in bb.instructions:
                si = ins.sync_info
                waits = list(si.on_wait) if (si is not None and si.on_wait) else []
                if len(waits) > max_waits:
                    extra, keep = waits[:-max_waits], waits[-max_waits:]
                    for w in extra:
                        out.append(
                            mybir.InstEventSemaphore(
                                name=f"I-{nc.next_id()}",
                                engine=ins.engine,
                                ins=[],
                                outs=[],
                                sync_info=mybir.SyncInfo(on_wait=[w], on_update=[]),
                            )
                        )
                    si.on_wait = keep
                    changed = True
                out.append(ins)
            if changed:
                bb.instructions = out


def _build_program(repeat=1, legalize=True):
    A = mybir.AluOpType
    nc = bass.Bass("TRN2", debug=False, target_bir_lowering=False)

    lhst = nc.dram_tensor("lhst", [KA, NQ], F32, kind="ExternalInput").ap()
    rhs = nc.dram_tensor("rhs", [KA, M], F32, kind="ExternalInput").ap()
    # slot-reordered reference table: ftab[s, m] = row of original column
    # s + (M//4)*m, fields [y0, y1, y2, r2, f0, f1, f2, 0]
    ftab = nc.dram_tensor("ftab", [M // NMEM, NMEM * 8], F32,
                          kind="ExternalInput").ap()
    xq = nc.dram_tensor("xq", [NQ, 3], F32, kind="ExternalInput").ap()
    q2q = nc.dram_tensor("q2q", [NQ], F32, kind="ExternalInput").ap()
    iot = nc.dram_tensor("iot", [P, NCAND], F32, kind="ExternalInput").ap()
    outf = nc.dram_tensor("outf", [NQ, 3], F32, kind="ExternalOutput").ap()

    with tile.TileContext(nc) as tc, ExitStack() as ctx:
        const_pool = ctx.enter_context(tc.tile_pool(name="const", bufs=1))
        negd2_pool = ctx.enter_context(tc.tile_pool(name="negd2", bufs=3))
        fold_pool = ctx.enter_context(tc.tile_pool(name="fold", bufs=2))
        psum_pool = ctx.enter_context(tc.tile_pool(name="psum", bufs=2, space="PSUM"))
        small_pool = ctx.enter_context(tc.tile_pool(name="small", bufs=3))

        # operands replicated at partition bases 0/32/64/96 for 4-wide
        # PE row-group packing (tile_position)
        rhs_rep = const_pool.tile([96 + KA, M], F32)
        lhst_rep = const_pool.tile([96 + KA, NQ], F32)
        for j in range(4):
            nc.sync.dma_start(lhst_rep[32 * j:32 * j + KA, :], lhst[:])
            nc.sync.dma_start(rhs_rep[32 * j:32 * j + KA, :], rhs[:])
        # query coords / norms as per-partition scalars: [128, NT, ...]
        xq_sb = const_pool.tile([P, NT, 3], F32)
        nc.sync.dma_start(xq_sb[:], xq.rearrange("(t p) c -> p t c", p=P))
        q2_sb = const_pool.tile([P, NT], F32)
        nc.sync.dma_start(q2_sb[:], q2q.rearrange("(t p) -> p t", p=P))
        iota_sb = const_pool.tile([P, NCAND], F32)
        nc.sync.dma_start(iota_sb[:], iot[:])

        def tile_body(t):
            # negd2[q, j] = 2 x_q . y_j - |x_q|^2 - |y_j|^2  (= -d2), bf16.
            # 4 matmuls run concurrently in separate PE row groups and land in
            # one 4-bank PSUM tile, evacuated with a single big ACT cast-copy.
            negd2 = negd2_pool.tile([P, M], BF16)
            for c4 in range(NCH // 4):
                ps = psum_pool.tile([P, 4 * CH], F32)
                for j in range(4):
                    c = 4 * c4 + j
                    nc.tensor.matmul(
                        ps[:, j * CH:(j + 1) * CH],
                        lhst_rep[32 * j:32 * j + KA, t * P:(t + 1) * P],
                        rhs_rep[32 * j:32 * j + KA, c * CH:(c + 1) * CH],
                        start=True,
                        stop=True,
                        tile_position=(32 * j, 0),
                    )
                nc.scalar.copy(
                    negd2[:, c4 * 4 * CH:(c4 + 1) * 4 * CH], ps[:]
                )

            # bf16 max-fold levels: 8192 -> 4096 -> 2048 -> 1024
            mx1 = fold_pool.tile([P, M // 2], BF16)
            nc.vector.tensor_tensor(
                mx1[:], negd2[:, :M // 2], negd2[:, M // 2:], op=A.max
            )
            mx2 = fold_pool.tile([P, M // 4], BF16)
            nc.vector.tensor_tensor(
                mx2[:], mx1[:, :M // 4], mx1[:, M // 4:], op=A.max
            )
            mx3 = fold_pool.tile([P, M // 8], BF16)
            nc.vector.tensor_tensor(
                mx3[:], mx2[:, :M // 8], mx2[:, M // 8:], op=A.max
            )

            # top fold slots (descending) + their positions
            ftop8 = small_pool.tile([P, 8], BF16)
            nc.vector.max(ftop8[:], mx3[:])
            slot8 = small_pool.tile([P, 8], U32)
            nc.vector.max_index(slot8[:], ftop8[:], mx3[:])

            # gather each top slot's 4 candidate rows (one 128B row per slot).
            # The HW indirect DMA needs an offset-0 destination AP, so gather
            # into dedicated tiles and consolidate with small copies.
            gdat = small_pool.tile([P, NSLOT, NMEM, 8], F32)
            for k in range(NSLOT):
                gk = small_pool.tile([P, NMEM * 8], F32, tag=f"gk{k}")
                nc.gpsimd.indirect_dma_start(
                    out=gk[:],
                    out_offset=None,
                    in_=ftab[:],
                    in_offset=bass.IndirectOffsetOnAxis(
                        ap=slot8[:, k:k + 1], axis=0
                    ),
                )
                nc.vector.tensor_copy(
                    gdat[:, k, :, :].rearrange("p a b -> p (a b)"), gk[:]
                )

            # recompute exact fp32 -d2 for the candidates:
            #   negd2c = (2 * (x . y) - q2) - r2
            x0 = xq_sb[:, t, 0:1]
            x1 = xq_sb[:, t, 1:2]
            x2 = xq_sb[:, t, 2:3]
            q2s = q2_sb[:, t:t + 1]
            cr = small_pool.tile([P, NSLOT, NMEM], F32)
            nc.gpsimd.tensor_scalar_mul(cr[:], gdat[:, :, :, 0], x0)
            m1 = small_pool.tile([P, NSLOT, NMEM], F32)
            nc.gpsimd.tensor_scalar_mul(m1[:], gdat[:, :, :, 1], x1)
            cr2 = small_pool.tile([P, NSLOT, NMEM], F32)
            nc.gpsimd.tensor_tensor(cr2[:], cr[:], m1[:], op=A.add)
            m2 = small_pool.tile([P, NSLOT, NMEM], F32)
            nc.gpsimd.tensor_scalar_mul(m2[:], gdat[:, :, :, 2], x2)
            cr3 = small_pool.tile([P, NSLOT, NMEM], F32)
            nc.gpsimd.tensor_tensor(cr3[:], cr2[:], m2[:], op=A.add)
            u = small_pool.tile([P, NSLOT, NMEM], F32)
            nc.gpsimd.tensor_scalar(
                u[:], cr3[:], 2.0, q2s, op0=A.mult, op1=A.subtract
            )
            negd2c = small_pool.tile([P, NSLOT, NMEM], F32)
            nc.gpsimd.tensor_tensor(
                negd2c[:], u[:], gdat[:, :, :, 3], op=A.subtract
            )

            # exact top-3 of the candidates (+ positions; positions are
            # distinct even for exactly-tied values, matching top_k)
            top8 = small_pool.tile([P, 8], F32)
            nc.vector.max(top8[:], negd2c[:])
            pos8 = small_pool.tile([P, 8], U32)
            nc.vector.max_index(
                pos8[:], top8[:], negd2c[:].rearrange("p a b -> p (a b)")
            )
            pos8f = small_pool.tile([P, 8], F32)
            nc.vector.tensor_copy(pos8f[:], pos8[:])

            # d = sqrt(max(d2, 0)); w = 1/(d + eps); w /= sum(w)
            nd3 = small_pool.tile([P, 3], F32)
            nc.gpsimd.tensor_scalar_min(nd3[:], top8[:, 0:3], 0.0)
            d3 = small_pool.tile([P, 3], F32)
            nc.scalar.activation(
                d3[:], nd3[:], mybir.ActivationFunctionType.Sqrt, scale=-1.0
            )
            dp = small_pool.tile([P, 3], F32)
            nc.gpsimd.tensor_scalar_add(dp[:], d3[:], EPS)
            w = small_pool.tile([P, 3], F32)
            nc.vector.reciprocal(w[:], dp[:])
            wcp = small_pool.tile([P, 3], F32)
            wsum = small_pool.tile([P, 1], F32)
            nc.scalar.activation(
                wcp[:], w[:], mybir.ActivationFunctionType.Identity,
                accum_out=wsum[:],
            )
            winv = small_pool.tile([P, 1], F32)
            nc.vector.reciprocal(winv[:], wsum[:])
            wn = small_pool.tile([P, 3], F32)
            nc.scalar.mul(wn[:], w[:], winv[:, 0:1])

            # Wc[j] = sum_k wn_k * [j == pos_k]  (positional one-hot weights)
            wca = small_pool.tile([P, NCAND], F32)
            nc.vector.tensor_scalar(
                wca[:], iota_sb[:], pos8f[:, 0:1], wn[:, 0:1],
                op0=A.is_equal, op1=A.mult,
            )
            wcb = small_pool.tile([P, NCAND], F32)
            nc.vector.tensor_scalar(
                wcb[:], iota_sb[:], pos8f[:, 1:2], wn[:, 1:2],
                op0=A.is_equal, op1=A.mult,
            )
            wcc = small_pool.tile([P, NCAND], F32)
            nc.vector.tensor_scalar(
                wcc[:], iota_sb[:], pos8f[:, 2:3], wn[:, 2:3],
                op0=A.is_equal, op1=A.mult,
            )
            wsum2 = small_pool.tile([P, NCAND], F32)
            nc.vector.tensor_tensor(wsum2[:], wca[:], wcb[:], op=A.add)
            wall = small_pool.tile([P, NCAND], F32)
            nc.vector.tensor_tensor(wall[:], wsum2[:], wcc[:], op=A.add)

            # flow_out[c] = sum_j Wc[j] * flow_c[j]  (fused mult+row-sum)
            acc = small_pool.tile([P, 3], F32)
            junk = small_pool.tile([P, NCAND], F32)
            wallv = wall[:].rearrange("p (a b) -> p a b", a=NSLOT)
            for c in range(3):
                nc.vector.scalar_tensor_tensor(
                    junk[:].rearrange("p (a b) -> p a b", a=NSLOT),
                    wallv, 1.0, gdat[:, :, :, 4 + c],
                    op0=A.bypass, op1=A.mult,
                    accum_out=acc[:, c:c + 1],
                )
            nc.sync.dma_start(outf[t * P:(t + 1) * P, :], acc[:])

        if repeat > 1:
            with tc.For_i(0, repeat, 1):
                for t in range(NT):
                    tile_body(t)
        else:
            for t in range(NT):
                tile_body(t)

    if legalize:
        _legalize_wait_counts(nc)
    return nc


def _get_nc():
    if "nc" not in _CACHE:
        _CACHE["nc"] = _build_program()
    return _CACHE["nc"]


def _host_prep(pc0, pc1, flow1, pose0, pose1):
    """fp32 pose math + augmented operand construction (matches reference)."""
    pc0 = np.asarray(pc0, dtype=np.float32)
    pc1 = np.asarray(pc1, dtype=np.float32)
    flow1 = np.asarray(flow1, dtype=np.float32)
    pose0 = np.asarray(pose0, dtype=np.float32)
    pose1 = np.asarray(pose1, dtype=np.float32)

    pose_0to1 = (np.linalg.inv(pose1) @ pose0).astype(np.float32)
    R = pose_0to1[:, :3, :3]
    t = pose_0to1[:, :3, 3]
    pc0_t = (np.einsum("bij,bnj->bni", R, pc0) + t[:, None, :]).astype(np.float32)
    pose_flow = pc0_t - pc0

    q2 = np.sum(pc0_t * pc0_t, axis=-1)  # [B, N]
    r2 = np.sum(pc1 * pc1, axis=-1)      # [B, M]

    in_maps = []
    for core in range(NCORES):
        b, s = divmod(core, SHARDS)
        sl = slice(s * NQ, (s + 1) * NQ)
        x = pc0_t[b, sl]                          # [NQ, 3]
        lhst = np.empty((KA, NQ), np.float32)
        lhst[0:3] = (2.0 * x).T
        lhst[3] = -q2[b, sl]
        lhst[4] = -1.0
        rhs = np.empty((KA, M), np.float32)
        rhs[0:3] = pc1[b].T
        rhs[3] = 1.0
        rhs[4] = r2[b]
        # slot-reordered table: row s, member m = original column s+2048m
        base = np.zeros((M, 8), np.float32)
        base[:, 0:3] = pc1[b]
        base[:, 3] = r2[b]
        base[:, 4:7] = flow1[b]
        ftab = np.ascontiguousarray(
            base.reshape(NMEM, M // NMEM, 8).transpose(1, 0, 2)
        ).reshape(M // NMEM, NMEM * 8)
        in_maps.append({
            "lhst": lhst, "rhs": rhs, "ftab": ftab,
            "xq": np.ascontiguousarray(x),
            "q2q": np.ascontiguousarray(q2[b, sl]),
            "iot": np.tile(np.arange(NCAND, dtype=np.float32), (P, 1)),
        })
    return in_maps, pose_flow


def kernel(pc0, pc1, flow1, pose0, pose1):
    global LAST_RESULTS
    in_maps, pose_flow = _host_prep(pc0, pc1, flow1, pose0, pose1)
    nc = _get_nc()
    res = run_bass_kernel_spmd(nc, in_maps, list(range(NCORES)))
    LAST_RESULTS = res
    flow_interp = np.empty((B, N, 3), np.float32)
    for core in range(NCORES):
        b, s = divmod(core, SHARDS)
        flow_interp[b, s * NQ:(s + 1) * NQ] = res.results[core]["outf"]
    return flow_interp, pose_flow


# revision 30
# speedup vs baseline: 1541.6179x; 1541.6179x over previous
"""KNN flow interpolation (AccFlowSupervise) on 8 Trainium2 NeuronCores.

Problem: for each query point (pc0 transformed into pc1's ego frame), find the
3 nearest neighbors in pc1, inverse-distance-weight their flow vectors, and
also emit the ego-motion displacement (pose_flow).

Sharding: data-parallel over batch (B=2) x 4 query shards -> 8 cores. Each
core computes a [2048, 8192] negated-squared-distance matrix against the full
replicated reference cloud via fused PE matmuls (augmented K=5 inner dim:
-d2 = 2x.y - |x|^2 - |y|^2), packed 4-wide into PE row groups
(tile_position), then cast to bf16 on the scalar engine.

Top-3 selection per query tile [128 x 8192]:
  1. two bf16 max-fold levels (8192 -> 4096 -> 2048) on the vector engine
     (bf16 tensor_tensor runs in 2x mode);
  2. top-8 fold slots via InstMax + InstMaxIndex on the 2048-wide folded
     array; each slot covers 4 original columns. The true top-3's slots rank
     <= 3 by exact folded value (a slot's fold >= its members); 5 slots are
     kept as margin for bf16 rounding ties.
  3. one indirect DMA per slot gathers that slot's 4 candidate rows from a
     host-reordered table (ftab[s, m] = [y, |y|^2, flow, 0] of original
     column s + 2048 m) - 20 candidates;
  4. exact fp32 -d2 recompute for the candidates on GPSIMD, exact top-3 of
     20 via a tiny InstMax/InstMaxIndex (positions stay distinct for tied
     values, matching top_k), inverse-distance weights, positional one-hot
     weighted flow combine.

The tiny pose math (4x4 inverse, [N,3] point transform, q2/r2 row norms and
pose_flow = pc0_t - pc0) is done host-side in fp32, matching the reference's
formulas; the O(N*M) work all runs on-device.
"""

import os
import sys
from contextlib import ExitStack

import numpy as np

for _p in ("/opt/trn_rl_repo", "/root/.axon_site/_ro/trn_rl_repo"):
    if os.path.isdir(_p) and _p not in sys.path:
        sys.path.append(_p)

import concourse.bass as bass
import concourse.tile as tile
from concourse import mybir
from concourse.bass_utils import run_bass_kernel_spmd

try:  # tracing needs the axon NTFF hook; disable it where the hook is absent
    import antenv.axon_hooks  # noqa: F401
except ImportError:
    os.environ["BASS_NEVER_TRACE"] = "1"

# Problem shape (hardcoded; see spec)
B, N, M = 2, 8192, 8192
NCORES = 8
SHARDS = NCORES // B          # query shards per batch
NQ = N // SHARDS              # queries per core
P = 128                       # queries per tile (SBUF partitions)
NT = NQ // P                  # query tiles per core
CH = 512                      # matmul free-dim chunk (one PSUM bank fp32)
NCH = M // CH                 # chunks per tile
KA = 5                        # augmented contraction dim
NFOLD = 4                     # max-fold levels (8192 -> 512)
NSLOT = 4                     # fold slots examined per query
NMEM = 1 << NFOLD             # original columns per fold slot
NCAND = NSLOT * NMEM          # candidates per query
MW = M >> NFOLD               # folded width
F32 = mybir.dt.float32
BF16 = mybir.dt.bfloat16
U32 = mybir.dt.uint32
EPS = 1e-8

_CACHE: dict = {}
LAST_RESULTS = None  # BassKernelResults of the most recent run (for profiling)


def _patched_drain_and_barrier(self, tick_clock, wait_clock):
    """Tile's kernel-tail drain can accumulate >2 sem waits on one Drain
    instruction, which this walrus build rejects ("Too many sync wait
    commands"). Split the waits across a chain of single-wait drains."""
    nc = self.nc
    drain_inst = nc.sync.drain()
    wait_clock.add_sem_waits(
        drain_inst.ins, tile.ScopedClock({None: tick_clock.global_clock})
    )
    si = drain_inst.ins.sync_info
    waits = list(si.on_wait or []) if si is not None else []
    if len(waits) > 1:
        si.on_wait = waits[:1]
        for w in waits[1:]:
            d2 = nc.sync.drain()
            d2.ins.sync_info = mybir.SyncInfo(on_wait=[w], on_update=[])
    nc.all_engine_barrier()
    assert self.sems is not None
    popped = nc._tile_sem_poison_stack.pop()
    assert popped is self._sem_poison
    nc.clear_and_free_semaphores(list(self.sems.allocated().values()))
    nc.all_engine_barrier()


tile.TileContext._drain_and_barrier = _patched_drain_and_barrier


def _legalize_wait_counts(nc, max_waits=1):
    """This walrus build rejects instructions carrying more than a couple of
    sem waits ("Too many sync wait commands"). Hoist extra waits onto fresh
    same-engine EventSemaphore carriers placed immediately before the
    instruction (same engine queue => identical ordering semantics)."""
    for fn in nc.m.functions:
        for bb in fn.blocks:
            out = []
            changed = False
            for ins in bb.instructions:
                si = ins.sync_info
                waits = list(si.on_wait) if (si is not None and si.on_wait) else []
                if len(waits) > max_waits:
                    extra, keep = waits[:-max_waits], waits[-max_waits:]
                    for w in extra:
                        out.append(
                            mybir.InstEventSemaphore(
                                name=f"I-{nc.next_id()}",
                                engine=ins.engine,
                                ins=[],
                                outs=[],
                                sync_info=mybir.SyncInfo(on_wait=[w], on_update=[]),
                            )
                        )
                    si.on_wait = keep
                    changed = True
                out.append(ins)
            if changed:
                bb.instructions = out


def _build_program(repeat=1, legalize=True):
    A = mybir.AluOpType
    nc = bass.Bass("TRN2", debug=False, target_bir_lowering=False)

    lhst = nc.dram_tensor("lhst", [KA, NQ], F32, kind="ExternalInput").ap()
    rhs = nc.dram_tensor("rhs", [KA, M], F32, kind="ExternalInput").ap()
    # slot-reordered reference table: ftab[s, m] = row of original column
    # s + (M//4)*m, fields [y0, y1, y2, r2, f0, f1, f2, 0]
    ftab = nc.dram_tensor("ftab", [M // NMEM, NMEM * 8], F32,
                          kind="ExternalInput").ap()
    xq = nc.dram_tensor("xq", [NQ, 3], F32, kind="ExternalInput").ap()
    q2q = nc.dram_tensor("q2q", [NQ], F32, kind="ExternalInput").ap()
    iot = nc.dram_tensor("iot", [P, NCAND], F32, kind="ExternalInput").ap()
    outf = nc.dram_tensor("outf", [NQ, 3], F32, kind="ExternalOutput").ap()

    with tile.TileContext(nc) as tc, ExitStack() as ctx:
        const_pool = ctx.enter_context(tc.tile_pool(name="const", bufs=1))
        negd2_pool = ctx.enter_context(tc.tile_pool(name="negd2", bufs=3))
        fold_pool = ctx.enter_context(tc.tile_pool(name="fold", bufs=2))
        psum_pool = ctx.enter_context(tc.tile_pool(name="psum", bufs=2, space="PSUM"))
        small_pool = ctx.enter_context(tc.tile_pool(name="small", bufs=3))

        # operands replicated at partition bases 0/32/64/96 for 4-wide
        # PE row-group packing (tile_position)
        rhs_rep = const_pool.tile([96 + KA, M], F32)
        lhst_rep = const_pool.tile([96 + KA, NQ], F32)
        for j in range(4):
            nc.sync.dma_start(lhst_rep[32 * j:32 * j + KA, :], lhst[:])
            nc.sync.dma_start(rhs_rep[32 * j:32 * j + KA, :], rhs[:])
        # query coords / norms as per-partition scalars: [128, NT, ...]
        xq_sb = const_pool.tile([P, NT, 3], F32)
        nc.sync.dma_start(xq_sb[:], xq.rearrange("(t p) c -> p t c", p=P))
        q2_sb = const_pool.tile([P, NT], F32)
        nc.sync.dma_start(q2_sb[:], q2q.rearrange("(t p) -> p t", p=P))
        iota_sb = const_pool.tile([P, NCAND], F32)
        nc.sync.dma_start(iota_sb[:], iot[:])

        def tile_body(t):
            # negd2[q, j] = 2 x_q . y_j - |x_q|^2 - |y_j|^2  (= -d2), bf16.
            # 4 matmuls run concurrently in separate PE row groups and land in
            # one 4-bank PSUM tile, evacuated with a single big ACT cast-copy.
            negd2 = negd2_pool.tile([P, M], BF16)
            for c4 in range(NCH // 4):
                ps = psum_pool.tile([P, 4 * CH], F32)
                for j in range(4):
                    c = 4 * c4 + j
                    nc.tensor.matmul(
                        ps[:, j * CH:(j + 1) * CH],
                        lhst_rep[32 * j:32 * j + KA, t * P:(t + 1) * P],
                        rhs_rep[32 * j:32 * j + KA, c * CH:(c + 1) * CH],
                        start=True,
                        stop=True,
                        tile_position=(32 * j, 0),
                    )
                nc.scalar.copy(
                    negd2[:, c4 * 4 * CH:(c4 + 1) * 4 * CH], ps[:]
                )

            # bf16 max-fold levels: 8192 -> 4096 -> 2048 -> 1024
            mx1 = fold_pool.tile([P, M // 2], BF16)
            nc.vector.tensor_tensor(
                mx1[:], negd2[:, :M // 2], negd2[:, M // 2:], op=A.max
            )
            mx2 = fold_pool.tile([P, M // 4], BF16)
            nc.vector.tensor_tensor(
                mx2[:], mx1[:, :M // 4], mx1[:, M // 4:], op=A.max
            )
            mx3 = fold_pool.tile([P, M // 8], BF16)
            nc.vector.tensor_tensor(
                mx3[:], mx2[:, :M // 8], mx2[:, M // 8:], op=A.max
            )
            mx4 = fold_pool.tile([P, M // 16], BF16)
            nc.vector.tensor_tensor(
                mx4[:], mx3[:, :M // 16], mx3[:, M // 16:], op=A.max
            )

            # top fold slots (descending) + their positions
            ftop8 = small_pool.tile([P, 8], BF16)
            nc.vector.max(ftop8[:], mx4[:])
            slot8 = small_pool.tile([P, 8], U32)
            nc.vector.max_index(slot8[:], ftop8[:], mx4[:])

            # gather each top slot's 4 candidate rows (one 128B row per slot).
            # The HW indirect DMA needs an offset-0 destination AP, so gather
            # into dedicated tiles and consolidate with small copies.
            gdat = small_pool.tile([P, NSLOT, NMEM, 8], F32)
            for k in range(NSLOT):
                gk = small_pool.tile([P, NMEM * 8], F32, tag=f"gk{k}")
                nc.gpsimd.indirect_dma_start(
                    out=gk[:],
                    out_offset=None,
                    in_=ftab[:],
                    in_offset=bass.IndirectOffsetOnAxis(
                        ap=slot8[:, k:k + 1], axis=0
                    ),
                )
                nc.vector.tensor_copy(
                    gdat[:, k, :, :].rearrange("p a b -> p (a b)"), gk[:]
                )

            # recompute exact fp32 -d2 for the candidates:
            #   negd2c = (2 * (x . y) - q2) - r2
            x0 = xq_sb[:, t, 0:1]
            x1 = xq_sb[:, t, 1:2]
            x2 = xq_sb[:, t, 2:3]
            q2s = q2_sb[:, t:t + 1]
            cr = small_pool.tile([P, NSLOT, NMEM], F32)
            nc.gpsimd.tensor_scalar_mul(cr[:], gdat[:, :, :, 0], x0)
            m1 = small_pool.tile([P, NSLOT, NMEM], F32)
            nc.gpsimd.tensor_scalar_mul(m1[:], gdat[:, :, :, 1], x1)
            cr2 = small_pool.tile([P, NSLOT, NMEM], F32)
            nc.gpsimd.tensor_tensor(cr2[:], cr[:], m1[:], op=A.add)
            m2 = small_pool.tile([P, NSLOT, NMEM], F32)
            nc.gpsimd.tensor_scalar_mul(m2[:], gdat[:, :, :, 2], x2)
            cr3 = small_pool.tile([P, NSLOT, NMEM], F32)
            nc.gpsimd.tensor_tensor(cr3[:], cr2[:], m2[:], op=A.add)
            u = small_pool.tile([P, NSLOT, NMEM], F32)
            nc.gpsimd.tensor_scalar(
                u[:], cr3[:], 2.0, q2s, op0=A.mult, op1=A.subtract
            )
            negd2c = small_pool.tile([P, NSLOT, NMEM], F32)
            nc.gpsimd.tensor_tensor(
                negd2c[:], u[:], gdat[:, :, :, 3], op=A.subtract
            )

            # exact top-3 of the candidates (+ positions; positions are
            # distinct even for exactly-tied values, matching top_k)
            top8 = small_pool.tile([P, 8], F32)
            nc.vector.max(top8[:], negd2c[:])
            pos8 = small_pool.tile([P, 8], U32)
            nc.vector.max_index(
                pos8[:], top8[:], negd2c[:].rearrange("p a b -> p (a b)")
            )
            pos8f = small_pool.tile([P, 8], F32)
            nc.vector.tensor_copy(pos8f[:], pos8[:])

            # d = sqrt(max(d2, 0)); w = 1/(d + eps); w /= sum(w)
            nd3 = small_pool.tile([P, 3], F32)
            nc.gpsimd.tensor_scalar_min(nd3[:], top8[:, 0:3], 0.0)
            d3 = small_pool.tile([P, 3], F32)
            nc.scalar.activation(
                d3[:], nd3[:], mybir.ActivationFunctionType.Sqrt, scale=-1.0
            )
            dp = small_pool.tile([P, 3], F32)
            nc.gpsimd.tensor_scalar_add(dp[:], d3[:], EPS)
            w = small_pool.tile([P, 3], F32)
            nc.vector.reciprocal(w[:], dp[:])
            wcp = small_pool.tile([P, 3], F32)
            wsum = small_pool.tile([P, 1], F32)
            nc.scalar.activation(
                wcp[:], w[:], mybir.ActivationFunctionType.Identity,
                accum_out=wsum[:],
            )
            winv = small_pool.tile([P, 1], F32)
            nc.vector.reciprocal(winv[:], wsum[:])
            wn = small_pool.tile([P, 3], F32)
            nc.scalar.mul(wn[:], w[:], winv[:, 0:1])

            # Wc[j] = sum_k wn_k * [j == pos_k]  (positional one-hot weights)
            wca = small_pool.tile([P, NCAND], F32)
            nc.vector.tensor_scalar(
                wca[:], iota_sb[:], pos8f[:, 0:1], wn[:, 0:1],
                op0=A.is_equal, op1=A.mult,
            )
            wcb = small_pool.tile([P, NCAND], F32)
            nc.vector.tensor_scalar(
                wcb[:], iota_sb[:], pos8f[:, 1:2], wn[:, 1:2],
                op0=A.is_equal, op1=A.mult,
            )
            wcc = small_pool.tile([P, NCAND], F32)
            nc.vector.tensor_scalar(
                wcc[:], iota_sb[:], pos8f[:, 2:3], wn[:, 2:3],
                op0=A.is_equal, op1=A.mult,
            )
            wsum2 = small_pool.tile([P, NCAND], F32)
            nc.vector.tensor_tensor(wsum2[:], wca[:], wcb[:], op=A.add)
            wall = small_pool.tile([P, NCAND], F32)
            nc.vector.tensor_tensor(wall[:], wsum2[:], wcc[:], op=A.add)

            # flow_out[c] = sum_j Wc[j] * flow_c[j]  (fused mult+row-sum)
            acc = small_pool.tile([P, 3], F32)
            junk = small_pool.tile([P, NCAND], F32)
            wallv = wall[:].rearrange("p (a b) -> p a b", a=NSLOT)
            for c in range(3):
                nc.vector.scalar_tensor_tensor(
                    junk[:].rearrange("p (a b) -> p a b", a=NSLOT),
                    wallv, 1.0, gdat[:, :, :, 4 + c],
                    op0=A.bypass, op1=A.mult,
                    accum_out=acc[:, c:c + 1],
                )
            nc.sync.dma_start(outf[t * P:(t + 1) * P, :], acc[:])

        if repeat > 1:
            with tc.For_i(0, repeat, 1):
                for t in range(NT):
                    tile_body(t)
        else:
            for t in range(NT):
                tile_body(t)

    if legalize:
        _legalize_wait_counts(nc)
    return nc


def _get_nc():
    if "nc" not in _CACHE:
        _CACHE["nc"] = _build_program()
    return _CACHE["nc"]


def _host_prep(pc0, pc1, flow1, pose0, pose1):
    """fp32 pose math + augmented operand construction (matches reference)."""
    pc0 = np.asarray(pc0, dtype=np.float32)
    pc1 = np.asarray(pc1, dtype=np.float32)
    flow1 = np.asarray(flow1, dtype=np.float32)
    pose0 = np.asarray(pose0, dtype=np.float32)
    pose1 = np.asarray(pose1, dtype=np.float32)

    pose_0to1 = (np.linalg.inv(pose1) @ pose0).astype(np.float32)
    R = pose_0to1[:, :3, :3]
    t = pose_0to1[:, :3, 3]
    pc0_t = (np.einsum("bij,bnj->bni", R, pc0) + t[:, None, :]).astype(np.float32)
    pose_flow = pc0_t - pc0

    q2 = np.sum(pc0_t * pc0_t, axis=-1)  # [B, N]
    r2 = np.sum(pc1 * pc1, axis=-1)      # [B, M]

    in_maps = []
    for core in range(NCORES):
        b, s = divmod(core, SHARDS)
        sl = slice(s * NQ, (s + 1) * NQ)
        x = pc0_t[b, sl]                          # [NQ, 3]
        lhst = np.empty((KA, NQ), np.float32)
        lhst[0:3] = (2.0 * x).T
        lhst[3] = -q2[b, sl]
        lhst[4] = -1.0
        rhs = np.empty((KA, M), np.float32)
        rhs[0:3] = pc1[b].T
        rhs[3] = 1.0
        rhs[4] = r2[b]
        # slot-reordered table: row s, member m = original column s+2048m
        base = np.zeros((M, 8), np.float32)
        base[:, 0:3] = pc1[b]
        base[:, 3] = r2[b]
        base[:, 4:7] = flow1[b]
        ftab = np.ascontiguousarray(
            base.reshape(NMEM, M // NMEM, 8).transpose(1, 0, 2)
        ).reshape(M // NMEM, NMEM * 8)
        in_maps.append({
            "lhst": lhst, "rhs": rhs, "ftab": ftab,
            "xq": np.ascontiguousarray(x),
            "q2q": np.ascontiguousarray(q2[b, sl]),
            "iot": np.tile(np.arange(NCAND, dtype=np.float32), (P, 1)),
        })
    return in_maps, pose_flow


def kernel(pc0, pc1, flow1, pose0, pose1):
    global LAST_RESULTS
    in_maps, pose_flow = _host_prep(pc0, pc1, flow1, pose0, pose1)
    nc = _get_nc()
    res = run_bass_kernel_spmd(nc, in_maps, list(range(NCORES)))
    LAST_RESULTS = res
    flow_interp = np.empty((B, N, 3), np.float32)
    for core in range(NCORES):
        b, s = divmod(core, SHARDS)
        flow_interp[b, s * NQ:(s + 1) * NQ] = res.results[core]["outf"]
    return flow_interp, pose_flow


# revision 32
# speedup vs baseline: 1560.3400x; 1.0121x over previous
"""KNN flow interpolation (AccFlowSupervise) on 8 Trainium2 NeuronCores.

Problem: for each query point (pc0 transformed into pc1's ego frame), find the
3 nearest neighbors in pc1, inverse-distance-weight their flow vectors, and
also emit the ego-motion displacement (pose_flow).

Sharding: data-parallel over batch (B=2) x 4 query shards -> 8 cores. Each
core computes a [2048, 8192] negated-squared-distance matrix against the full
replicated reference cloud via fused PE matmuls (augmented K=5 inner dim:
-d2 = 2x.y - |x|^2 - |y|^2), packed 4-wide into PE row groups
(tile_position), then cast to bf16 on the scalar engine.

Top-3 selection per query tile [128 x 8192]:
  1. two bf16 max-fold levels (8192 -> 4096 -> 2048) on the vector engine
     (bf16 tensor_tensor runs in 2x mode);
  2. top-8 fold slots via InstMax + InstMaxIndex on the 2048-wide folded
     array; each slot covers 4 original columns. The true top-3's slots rank
     <= 3 by exact folded value (a slot's fold >= its members); 5 slots are
     kept as margin for bf16 rounding ties.
  3. one indirect DMA per slot gathers that slot's 4 candidate rows from a
     host-reordered table (ftab[s, m] = [y, |y|^2, flow, 0] of original
     column s + 2048 m) - 20 candidates;
  4. exact fp32 -d2 recompute for the candidates on GPSIMD, exact top-3 of
     20 via a tiny InstMax/InstMaxIndex (positions stay distinct for tied
     values, matching top_k), inverse-distance weights, positional one-hot
     weighted flow combine.

The tiny pose math (4x4 inverse, [N,3] point transform, q2/r2 row norms and
pose_flow = pc0_t - pc0) is done host-side in fp32, matching the reference's
formulas; the O(N*M) work all runs on-device.
"""

import os
import sys
from contextlib import ExitStack

import numpy as np

for _p in ("/opt/trn_rl_repo", "/root/.axon_site/_ro/trn_rl_repo"):
    if os.path.isdir(_p) and _p not in sys.path:
        sys.path.append(_p)

import concourse.bass as bass
import concourse.tile as tile
from concourse import mybir
from concourse.bass_utils import run_bass_kernel_spmd

try:  # tracing needs the axon NTFF hook; disable it where the hook is absent
    import antenv.axon_hooks  # noqa: F401
except ImportError:
    os.environ["BASS_NEVER_TRACE"] = "1"

# Problem shape (hardcoded; see spec)
B, N, M = 2, 8192, 8192
NCORES = 8
SHARDS = NCORES // B          # query shards per batch
NQ = N // SHARDS              # queries per core
P = 128                       # queries per tile (SBUF partitions)
NT = NQ // P                  # query tiles per core
CH = 512                      # matmul free-dim chunk (one PSUM bank fp32)
NCH = M // CH                 # chunks per tile
KA = 5                        # augmented contraction dim
NFOLD = 4                     # max-fold levels (8192 -> 512)
NSLOT = 4                     # fold slots examined per query
NMEM = 1 << NFOLD             # original columns per fold slot
NCAND = NSLOT * NMEM          # candidates per query
MW = M >> NFOLD               # folded width
F32 = mybir.dt.float32
BF16 = mybir.dt.bfloat16
U32 = mybir.dt.uint32
EPS = 1e-8

_CACHE: dict = {}
LAST_RESULTS = None  # BassKernelResults of the most recent run (for profiling)


def _patched_drain_and_barrier(self, tick_clock, wait_clock):
    """Tile's kernel-tail drain can accumulate >2 sem waits on one Drain
    instruction, which this walrus build rejects ("Too many sync wait
    commands"). Split the waits across a chain of single-wait drains."""
    nc = self.nc
    drain_inst = nc.sync.drain()
    wait_clock.add_sem_waits(
        drain_inst.ins, tile.ScopedClock({None: tick_clock.global_clock})
    )
    si = drain_inst.ins.sync_info
    waits = list(si.on_wait or []) if si is not None else []
    if len(waits) > 1:
        si.on_wait = waits[:1]
        for w in waits[1:]:
            d2 = nc.sync.drain()
            d2.ins.sync_info = mybir.SyncInfo(on_wait=[w], on_update=[])
    nc.all_engine_barrier()
    assert self.sems is not None
    popped = nc._tile_sem_poison_stack.pop()
    assert popped is self._sem_poison
    nc.clear_and_free_semaphores(list(self.sems.allocated().values()))
    nc.all_engine_barrier()


tile.TileContext._drain_and_barrier = _patched_drain_and_barrier


def _legalize_wait_counts(nc, max_waits=1):
    """This walrus build rejects instructions carrying more than a couple of
    sem waits ("Too many sync wait commands"). Hoist extra waits onto fresh
    same-engine EventSemaphore carriers placed immediately before the
    instruction (same engine queue => identical ordering semantics)."""
    for fn in nc.m.functions:
        for bb in fn.blocks:
            out = []
            changed = False
            for ins in bb.instructions:
                si = ins.sync_info
                waits = list(si.on_wait) if (si is not None and si.on_wait) else []
                if len(waits) > max_waits:
                    extra, keep = waits[:-max_waits], waits[-max_waits:]
                    for w in extra:
                        out.append(
                            mybir.InstEventSemaphore(
                                name=f"I-{nc.next_id()}",
                                engine=ins.engine,
                                ins=[],
                                outs=[],
                                sync_info=mybir.SyncInfo(on_wait=[w], on_update=[]),
                            )
                        )
                    si.on_wait = keep
                    changed = True
                out.append(ins)
            if changed:
                bb.instructions = out


def _build_program(repeat=1, legalize=True):
    A = mybir.AluOpType
    nc = bass.Bass("TRN2", debug=False, target_bir_lowering=False)

    lhst = nc.dram_tensor("lhst", [KA, NQ], F32, kind="ExternalInput").ap()
    rhs = nc.dram_tensor("rhs", [KA, M], F32, kind="ExternalInput").ap()
    # slot-reordered reference table: ftab[s, m] = row of original column
    # s + (M//4)*m, fields [y0, y1, y2, r2, f0, f1, f2, 0]
    ftab = nc.dram_tensor("ftab", [M // NMEM, NMEM * 8], F32,
                          kind="ExternalInput").ap()
    xq = nc.dram_tensor("xq", [NQ, 3], F32, kind="ExternalInput").ap()
    q2q = nc.dram_tensor("q2q", [NQ], F32, kind="ExternalInput").ap()
    iot = nc.dram_tensor("iot", [P, NCAND], F32, kind="ExternalInput").ap()
    outf = nc.dram_tensor("outf", [NQ, 3], F32, kind="ExternalOutput").ap()

    with tile.TileContext(nc) as tc, ExitStack() as ctx:
        const_pool = ctx.enter_context(tc.tile_pool(name="const", bufs=1))
        negd2_pool = ctx.enter_context(tc.tile_pool(name="negd2", bufs=3))
        fold_pool = ctx.enter_context(tc.tile_pool(name="fold", bufs=2))
        psum_pool = ctx.enter_context(tc.tile_pool(name="psum", bufs=2, space="PSUM"))
        small_pool = ctx.enter_context(tc.tile_pool(name="small", bufs=3))

        # operands replicated at partition bases 0/32/64/96 for 4-wide
        # PE row-group packing (tile_position)
        rhs_rep = const_pool.tile([96 + KA, M], F32)
        lhst_rep = const_pool.tile([96 + KA, NQ], F32)
        for j in range(4):
            nc.sync.dma_start(lhst_rep[32 * j:32 * j + KA, :], lhst[:])
            nc.sync.dma_start(rhs_rep[32 * j:32 * j + KA, :], rhs[:])
        # query coords / norms as per-partition scalars: [128, NT, ...]
        xq_sb = const_pool.tile([P, NT, 3], F32)
        nc.sync.dma_start(xq_sb[:], xq.rearrange("(t p) c -> p t c", p=P))
        q2_sb = const_pool.tile([P, NT], F32)
        nc.sync.dma_start(q2_sb[:], q2q.rearrange("(t p) -> p t", p=P))
        iota_sb = const_pool.tile([P, NCAND], F32)
        nc.sync.dma_start(iota_sb[:], iot[:])

        def tile_body(t):
            # negd2[q, j] = 2 x_q . y_j - |x_q|^2 - |y_j|^2  (= -d2), bf16.
            # 4 matmuls run concurrently in separate PE row groups and land in
            # one 4-bank PSUM tile, evacuated with a single big ACT cast-copy.
            negd2 = negd2_pool.tile([P, M], BF16)
            for c4 in range(NCH // 4):
                ps = psum_pool.tile([P, 4 * CH], F32)
                for j in range(4):
                    c = 4 * c4 + j
                    nc.tensor.matmul(
                        ps[:, j * CH:(j + 1) * CH],
                        lhst_rep[32 * j:32 * j + KA, t * P:(t + 1) * P],
                        rhs_rep[32 * j:32 * j + KA, c * CH:(c + 1) * CH],
                        start=True,
                        stop=True,
                        tile_position=(32 * j, 0),
                    )
                nc.scalar.copy(
                    negd2[:, c4 * 4 * CH:(c4 + 1) * 4 * CH], ps[:]
                )

            # bf16 max-fold levels: 8192 -> 4096 -> 2048 -> 1024
            mx1 = fold_pool.tile([P, M // 2], BF16)
            nc.vector.tensor_tensor(
                mx1[:], negd2[:, :M // 2], negd2[:, M // 2:], op=A.max
            )
            mx2 = fold_pool.tile([P, M // 4], BF16)
            nc.vector.tensor_tensor(
                mx2[:], mx1[:, :M // 4], mx1[:, M // 4:], op=A.max
            )
            mx3 = fold_pool.tile([P, M // 8], BF16)
            nc.vector.tensor_tensor(
                mx3[:], mx2[:, :M // 8], mx2[:, M // 8:], op=A.max
            )
            mx4 = fold_pool.tile([P, M // 16], BF16)
            nc.vector.tensor_tensor(
                mx4[:], mx3[:, :M // 16], mx3[:, M // 16:], op=A.max
            )

            # top fold slots (descending) + their positions
            ftop8 = small_pool.tile([P, 8], BF16)
            nc.vector.max(ftop8[:], mx4[:])
            slot8 = small_pool.tile([P, 8], U32)
            nc.vector.max_index(slot8[:], ftop8[:], mx4[:])

            # gather each top slot's 4 candidate rows (one 128B row per slot).
            # The HW indirect DMA needs an offset-0 destination AP, so gather
            # into dedicated tiles and consolidate with small copies.
            gdat = small_pool.tile([P, NSLOT, NMEM, 8], F32)
            for k in range(NSLOT):
                gk = small_pool.tile([P, NMEM * 8], F32, tag=f"gk{k}")
                nc.gpsimd.indirect_dma_start(
                    out=gk[:],
                    out_offset=None,
                    in_=ftab[:],
                    in_offset=bass.IndirectOffsetOnAxis(
                        ap=slot8[:, k:k + 1], axis=0
                    ),
                )
                nc.gpsimd.tensor_copy(
                    gdat[:, k, :, :].rearrange("p a b -> p (a b)"), gk[:]
                )

            # recompute exact fp32 -d2 for the candidates:
            #   negd2c = (2 * (x . y) - q2) - r2
            x0 = xq_sb[:, t, 0:1]
            x1 = xq_sb[:, t, 1:2]
            x2 = xq_sb[:, t, 2:3]
            q2s = q2_sb[:, t:t + 1]
            cr = small_pool.tile([P, NSLOT, NMEM], F32)
            nc.gpsimd.tensor_scalar_mul(cr[:], gdat[:, :, :, 0], x0)
            m1 = small_pool.tile([P, NSLOT, NMEM], F32)
            nc.gpsimd.tensor_scalar_mul(m1[:], gdat[:, :, :, 1], x1)
            cr2 = small_pool.tile([P, NSLOT, NMEM], F32)
            nc.gpsimd.tensor_tensor(cr2[:], cr[:], m1[:], op=A.add)
            m2 = small_pool.tile([P, NSLOT, NMEM], F32)
            nc.gpsimd.tensor_scalar_mul(m2[:], gdat[:, :, :, 2], x2)
            cr3 = small_pool.tile([P, NSLOT, NMEM], F32)
            nc.gpsimd.tensor_tensor(cr3[:], cr2[:], m2[:], op=A.add)
            u = small_pool.tile([P, NSLOT, NMEM], F32)
            nc.gpsimd.tensor_scalar(
                u[:], cr3[:], 2.0, q2s, op0=A.mult, op1=A.subtract
            )
            negd2c = small_pool.tile([P, NSLOT, NMEM], F32)
            nc.gpsimd.tensor_tensor(
                negd2c[:], u[:], gdat[:, :, :, 3], op=A.subtract
            )

            # exact top-3 of the candidates (+ positions; positions are
            # distinct even for exactly-tied values, matching top_k)
            top8 = small_pool.tile([P, 8], F32)
            nc.vector.max(top8[:], negd2c[:])
            pos8 = small_pool.tile([P, 8], U32)
            nc.vector.max_index(
                pos8[:], top8[:], negd2c[:].rearrange("p a b -> p (a b)")
            )
            pos8f = small_pool.tile([P, 8], F32)
            nc.vector.tensor_copy(pos8f[:], pos8[:])

            # d = sqrt(max(d2, 0)); w = 1/(d + eps); w /= sum(w)
            nd3 = small_pool.tile([P, 3], F32)
            nc.gpsimd.tensor_scalar_min(nd3[:], top8[:, 0:3], 0.0)
            d3 = small_pool.tile([P, 3], F32)
            nc.scalar.activation(
                d3[:], nd3[:], mybir.ActivationFunctionType.Sqrt, scale=-1.0
            )
            dp = small_pool.tile([P, 3], F32)
            nc.gpsimd.tensor_scalar_add(dp[:], d3[:], EPS)
            w = small_pool.tile([P, 3], F32)
            nc.vector.reciprocal(w[:], dp[:])
            wsum = small_pool.tile([P, 1], F32)
            nc.vector.tensor_reduce(
                wsum[:], w[:], axis=mybir.AxisListType.X, op=A.add
            )
            winv = small_pool.tile([P, 1], F32)
            nc.vector.reciprocal(winv[:], wsum[:])
            wn = small_pool.tile([P, 3], F32)
            nc.gpsimd.tensor_scalar_mul(wn[:], w[:], winv[:, 0:1])

            # Wc[j] = sum_k wn_k * [j == pos_k]  (positional one-hot weights)
            wca = small_pool.tile([P, NCAND], F32)
            nc.vector.tensor_scalar(
                wca[:], iota_sb[:], pos8f[:, 0:1], wn[:, 0:1],
                op0=A.is_equal, op1=A.mult,
            )
            wcb = small_pool.tile([P, NCAND], F32)
            nc.vector.tensor_scalar(
                wcb[:], iota_sb[:], pos8f[:, 1:2], wn[:, 1:2],
                op0=A.is_equal, op1=A.mult,
            )
            wcc = small_pool.tile([P, NCAND], F32)
            nc.vector.tensor_scalar(
                wcc[:], iota_sb[:], pos8f[:, 2:3], wn[:, 2:3],
                op0=A.is_equal, op1=A.mult,
            )
            wsum2 = small_pool.tile([P, NCAND], F32)
            nc.vector.tensor_tensor(wsum2[:], wca[:], wcb[:], op=A.add)
            wall = small_pool.tile([P, NCAND], F32)
            nc.vector.tensor_tensor(wall[:], wsum2[:], wcc[:], op=A.add)

            # flow_out[c] = sum_j Wc[j] * flow_c[j]  (fused mult+row-sum)
            acc = small_pool.tile([P, 3], F32)
            junk = small_pool.tile([P, NCAND], F32)
            wallv = wall[:].rearrange("p (a b) -> p a b", a=NSLOT)
            for c in range(3):
                nc.vector.scalar_tensor_tensor(
                    junk[:].rearrange("p (a b) -> p a b", a=NSLOT),
                    wallv, 1.0, gdat[:, :, :, 4 + c],
                    op0=A.bypass, op1=A.mult,
                    accum_out=acc[:, c:c + 1],
                )
            nc.sync.dma_start(outf[t * P:(t + 1) * P, :], acc[:])

        if repeat > 1:
            with tc.For_i(0, repeat, 1):
                for t in range(NT):
                    tile_body(t)
        else:
            for t in range(NT):
                tile_body(t)

    if legalize:
        _legalize_wait_counts(nc)
    return nc


def _get_nc():
    if "nc" not in _CACHE:
        _CACHE["nc"] = _build_program()
    return _CACHE["nc"]


def _host_prep(pc0, pc1, flow1, pose0, pose1):
    """fp32 pose math + augmented operand construction (matches reference)."""
    pc0 = np.asarray(pc0, dtype=np.float32)
    pc1 = np.asarray(pc1, dtype=np.float32)
    flow1 = np.asarray(flow1, dtype=np.float32)
    pose0 = np.asarray(pose0, dtype=np.float32)
    pose1 = np.asarray(pose1, dtype=np.float32)

    pose_0to1 = (np.linalg.inv(pose1) @ pose0).astype(np.float32)
    R = pose_0to1[:, :3, :3]
    t = pose_0to1[:, :3, 3]
    pc0_t = (np.einsum("bij,bnj->bni", R, pc0) + t[:, None, :]).astype(np.float32)
    pose_flow = pc0_t - pc0

    q2 = np.sum(pc0_t * pc0_t, axis=-1)  # [B, N]
    r2 = np.sum(pc1 * pc1, axis=-1)      # [B, M]

    in_maps = []
    for core in range(NCORES):
        b, s = divmod(core, SHARDS)
        sl = slice(s * NQ, (s + 1) * NQ)
        x = pc0_t[b, sl]                          # [NQ, 3]
        lhst = np.empty((KA, NQ), np.float32)
        lhst[0:3] = (2.0 * x).T
        lhst[3] = -q2[b, sl]
        lhst[4] = -1.0
        rhs = np.empty((KA, M), np.float32)
        rhs[0:3] = pc1[b].T
        rhs[3] = 1.0
        rhs[4] = r2[b]
        # slot-reordered table: row s, member m = original column s+2048m
        base = np.zeros((M, 8), np.float32)
        base[:, 0:3] = pc1[b]
        base[:, 3] = r2[b]
        base[:, 4:7] = flow1[b]
        ftab = np.ascontiguousarray(
            base.reshape(NMEM, M // NMEM, 8).transpose(1, 0, 2)
        ).reshape(M // NMEM, NMEM * 8)
        in_maps.append({
            "lhst": lhst, "rhs": rhs, "ftab": ftab,
            "xq": np.ascontiguousarray(x),
            "q2q": np.ascontiguousarray(q2[b, sl]),
            "iot": np.tile(np.arange(NCAND, dtype=np.float32), (P, 1)),
        })
    return in_maps, pose_flow


def kernel(pc0, pc1, flow1, pose0, pose1):
    global LAST_RESULTS
    in_maps, pose_flow = _host_prep(pc0, pc1, flow1, pose0, pose1)
    nc = _get_nc()
    res = run_bass_kernel_spmd(nc, in_maps, list(range(NCORES)))
    LAST_RESULTS = res
    flow_interp = np.empty((B, N, 3), np.float32)
    for core in range(NCORES):
        b, s = divmod(core, SHARDS)
        flow_interp[b, s * NQ:(s + 1) * NQ] = res.results[core]["outf"]
    return flow_interp, pose_flow
